# revision 21
# baseline (speedup 1.0000x reference)
"""BitLinear fake-quant GEMM on 8 TRN2 NeuronCores — fp8 DoubleRow edition.

Reference math:
  abs_mean  = mean(|W|);  thr = 0.7*abs_mean
  Wq        = sign(W) * (|W| >= thr)            (ternary)
  scale_w   = abs_mean / (mean(Wq != 0) + 1e-8)
  sx        = 127 / max(|X|)
  Xq        = round(X * sx)                      (integer valued, |.| <= 127)
  out       = (Xq @ Wq^T) * scale_w / sx

Sharding: data-parallel over tokens (8192/8 = 1024 token columns per core);
W is replicated.  The GEMM runs on the tensor engine in fp8e4 DoubleRow
perf mode, which fuses two K=128 plane-products into one matmul at half
the per-row cost.  The two planes carry an EXACT hi/lo split of the
integer activations:

  Xc = e4m3(Xq)          (RNE cast of the integer; |Xq| <= 127 < 240)
  R  = Xq - Xc           (integer residual, |R| <= 4 — exactly fp8)

so with the stationary weight plane pair (Wq, Wq) — a stride-0 broadcast,
no duplication — each DoubleRow matmul computes
  psum[o, t] += sum_i Wq[i,o]*(Xc[i,t] + R[i,t]) = sum_i Wq[i,o]*Xq[i,t]
exactly, in integers.

X is read once in fp32 (feeding the exact |x| max), cast to bf16 and kept
resident in SBUF; quantization re-reads it from SBUF.  The bf16 staging
perturbs round(x*sx) on ~2% of elements by +-1 (output max err ~2-3 vs a
tolerance of ~11), and the output is stored as bf16 (integer-valued sums
up to ~7000, max err ~1).  Everything else is exact.

Scheduling notes (engine SEQs are in-order with a 4-deep wait queue, and
the DMA device is a single 360 GB/s FIFO ordered by request time):
  SP   : x DMAs, then the collective staging hops (cin/gg) and stats
         outputs, then the out-chunk stores.  Nothing bulky is allowed to
         queue ahead of the staging hops.
  ACT  : x bf16 casts -> wsl DMA issues (their requests land right after
         the x stream drains) -> u = Id(sx*xbf + MAGIC) + half the Xc
         casts -> psum->bf16 output copies.
  Pool : x-max allreduce + AllGather (sx), panel-0 W-quarter DMA issues,
         w allreduce + AllGather (thr), panel 1+ W-quarter DMA issues
         (all W streaming stays off the gated SP/ACT queues).
  DVE  : x-max reduces, wsl reduces, X pair split (stt + half the Xc
         casts), W quant (b2 + signed combine) pipelined ahead of the PE.

Stats: each core reduces its own x shard and a distinct 512-row slice of
W^T; two tiny AllGathers + local reduce replace the global mean/max
all-reduces.  nnz falls out of the W-quant DVE ops' accum_out side-sums
(every core sees the full W).  The final scalar rescale by scale_w/sx is
applied on the host during the unshard, together with the [o, t] ->
[t, o] transpose of each shard.
"""

from contextlib import ExitStack

import numpy as np

import concourse.bass as bass
import concourse.bass_isa as bass_isa
import concourse.tile as tile
from concourse import bacc, mybir
from concourse.bass import ts as _ts
from concourse.bass_utils import run_bass_kernel_spmd

P = 128
T, I, O = 8192, 4096, 4096  # tokens, in_features, out_features
NC = 8
TSH = T // NC  # 1024 token columns per core
ISL = I // NC  # 512 wT rows per core for stats
NMM = 512  # matmul moving free dim (one fp32 PSUM bank)
GF = 4096  # streaming tile free size (one [128, 4096] fp32 tile = 2 MB)
MAGIC = 12582912.0  # 1.5 * 2**23: fp32 round-to-nearest-even bias trick

F32 = mybir.dt.float32
BF16 = mybir.dt.bfloat16
F8E4 = mybir.dt.float8e4
ALU = mybir.AluOpType
AXX = mybir.AxisListType
IDENT = mybir.ActivationFunctionType.Identity


def _bitlinear(tc, out, sout, xT, wT, wsl):
    nc = tc.nc
    with ExitStack() as ctx:
        const = ctx.enter_context(tc.tile_pool(name="const", bufs=1))
        statp = ctx.enter_context(tc.tile_pool(name="statp", bufs=1))
        dram = ctx.enter_context(tc.tile_pool(name="dram", bufs=1, space="DRAM"))
        stgx = ctx.enter_context(tc.tile_pool(name="stgx", bufs=2))   # f32 [128,4096]
        # shared pool: 8 resident bf16 x tiles, then 8 fp8 (Xc|R) pair tiles
        # reuse the freed slots (all 8 KB/partition)
        xmem = ctx.enter_context(tc.tile_pool(name="xmem", bufs=8))
        stgw = ctx.enter_context(tc.tile_pool(name="stgw", bufs=4))   # f32 [128,4096]
        wslp = ctx.enter_context(tc.tile_pool(name="wslp", bufs=4))   # f32 [128,512]
        b2p = ctx.enter_context(tc.tile_pool(name="b2p", bufs=1))     # f8 [128,4096]
        wqp = ctx.enter_context(tc.tile_pool(name="wqp", bufs=2))     # 4x f8 [128,4096]
        psum = ctx.enter_context(tc.tile_pool(name="psum", bufs=1, space="PSUM"))
        osb = ctx.enter_context(tc.tile_pool(name="osb", bufs=1))     # bf16 [128,512]

        pmagic128 = const.tile([P, 1], F32)
        nc.gpsimd.memset(pmagic128[:], MAGIC)
        nmagic128 = const.tile([P, 1], F32)
        nc.gpsimd.memset(nmagic128[:], -MAGIC)

        # ---- Phase 1a: x stream (SP), stats reduces (DVE), bf16 casts (ACT)
        xmax_part = statp.tile([P, 8], F32)
        xbf_tiles = [None] * 8
        for g in range(8):
            xt = stgx.tile([P, GF], F32, tag="xstage")
            src = xT[g * 512 : (g + 1) * 512, :].rearrange("(c p) t -> p c t", p=P)
            nc.sync.dma_start(xt[:].rearrange("p (c t) -> p c t", c=4), src)
            nc.vector.tensor_reduce(
                xmax_part[:, g : g + 1], xt[:], axis=AXX.X, op=ALU.max,
                apply_absolute_value=True,
            )
            xb = xmem.tile([P, GF], BF16, tag="xm", name=f"xb{g}")
            nc.scalar.copy(xb[:], xt[:])
            xbf_tiles[g] = xb

        # ---- Phase 1b: x-max collective staging.  The SP queue order is
        # [x DMAs, cin_x, wsl DMAs, ggx, cin_w, ggw]: the DMA device is a
        # FIFO ordered by request time, and each gated SP instruction
        # naturally holds the next bulk stream's requests back until the
        # preceding collective hop is in flight. ----
        xmax_c = statp.tile([P, 1], F32)
        nc.vector.tensor_reduce(xmax_c[:], xmax_part[:], axis=AXX.X, op=ALU.max)
        xmax_a = statp.tile([P, 1], F32)
        nc.gpsimd.partition_all_reduce(
            xmax_a[:], xmax_c[:], channels=P, reduce_op=bass_isa.ReduceOp.max
        )
        cin_x = dram.tile([1, 1], F32)
        cout_x = dram.tile([1, NC], F32)
        nc.sync.dma_start(cin_x[:], xmax_a[0:1, 0:1])
        nc.gpsimd.collective_compute(
            "AllGather", ALU.bypass, replica_groups=[list(range(NC))],
            ins=[cin_x.opt()], outs=[cout_x.opt()],
        )
        # wsl stream: SP issues these only after cin_x's wait fires, so
        # their requests queue behind the whole x stream + cin_x
        wsl_tiles = []
        for c in range(32):
            wt = wslp.tile([P, GF // 8], F32, tag="wslstage")
            nc.sync.dma_start(
                wt[:],
                wsl[(c // 8) * P : (c // 8 + 1) * P,
                    (c % 8) * (GF // 8) : (c % 8 + 1) * (GF // 8)],
            )
            wsl_tiles.append(wt)
        ggx = statp.tile([1, NC], F32)
        nc.sync.dma_start(ggx[:], cout_x[:])
        gmax = statp.tile([1, 1], F32)
        nc.vector.tensor_reduce(gmax[:], ggx[:], axis=AXX.X, op=ALU.max)
        gmax_c = statp.tile([1, 1], F32)
        nc.vector.tensor_scalar(gmax_c[:], gmax[:], 1e-12, None, op0=ALU.max)
        rec1 = statp.tile([1, 1], F32)
        nc.vector.reciprocal(rec1[:], gmax_c[:])
        sx1 = statp.tile([1, 1], F32)
        nc.vector.tensor_scalar(sx1[:], rec1[:], 127.0, None, op0=ALU.mult)
        sx128 = const.tile([P, 1], F32)
        nc.gpsimd.partition_broadcast(sx128[:], sx1[:])

        # ---- W quarter streaming (DMA on the Pool queue; quant on DVE) ----
        qaccs = statp.tile([P, 32], F32)  # sum(Wq) per quarter  ( #pos - #neg )
        naccs = statp.tile([P, 32], F32)  # sum(b2) per quarter  ( #neg )

        def quarter_dma(op_, q, engine=None):
            wt = stgw.tile([P, GF], F32, tag="wstage")
            src_ = wT[
                q * 1024 : (q + 1) * 1024, _ts(op_, NMM)
            ].rearrange("(c p) j -> p c j", p=P)
            (engine or nc.gpsimd).dma_start(
                wt[:].rearrange("p (c j) -> p c j", c=8), src_)
            return wt

        def quarter_quant(op_, q, wt):
            col = op_ * 4 + q
            b2 = b2p.tile([P, GF], F8E4, tag="b2")
            nc.vector.tensor_scalar(
                b2[:], wt[:], nthr128[:], None, op0=ALU.is_le, op1=ALU.add,
                accum_out=naccs[:, col : col + 1],
            )
            wq = wqp.tile([P, GF], F8E4, tag=f"wq{q}")
            nc.vector.scalar_tensor_tensor(
                wq[:], wt[:], thr128[:], b2[:],
                op0=ALU.is_ge, op1=ALU.subtract,
                accum_out=qaccs[:, col : col + 1],
            )
            return wq[:].rearrange("p (c j) -> p c j", c=8)

        # ---- wsl reduces (DVE) + W-sum collective.  Pool order:
        # [allred-x, allred-w, P0 quarter DMAs, bcast-sx, bcast-thr...]:
        # the P0 DMA issues fire right after allred-w (~when the wsl
        # stream drains), so their requests land between cin_w and the
        # Pool-streamed panel-1+ quarters. ----
        wsum_part = statp.tile([P, 32], F32)
        for c in range(32):
            nc.vector.tensor_reduce(
                wsum_part[:, c : c + 1], wsl_tiles[c][:], axis=AXX.X, op=ALU.add,
                apply_absolute_value=True,
            )
        wsum_c = statp.tile([P, 1], F32)
        nc.vector.tensor_reduce(wsum_c[:], wsum_part[:], axis=AXX.X, op=ALU.add)
        wsum_a = statp.tile([P, 1], F32)
        nc.gpsimd.partition_all_reduce(
            wsum_a[:], wsum_c[:], channels=P, reduce_op=bass_isa.ReduceOp.add
        )
        cin_w = dram.tile([1, 1], F32)
        cout_w = dram.tile([1, NC], F32)
        nc.sync.dma_start(cin_w[:], wsum_a[0:1, 0:1])
        nc.gpsimd.collective_compute(
            "AllGather", ALU.bypass, replica_groups=[list(range(NC))],
            ins=[cin_w.opt()], outs=[cout_w.opt()],
        )
        ggw = statp.tile([1, NC], F32)
        nc.sync.dma_start(ggw[:], cout_w[:])
        p0_tiles = [quarter_dma(0, q, engine=nc.sync) for q in range(4)]
        gsum = statp.tile([1, 1], F32)
        nc.vector.tensor_reduce(gsum[:], ggw[:], axis=AXX.X, op=ALU.add)
        thr1 = statp.tile([1, 1], F32)
        nc.vector.tensor_scalar(thr1[:], gsum[:], 0.7 / float(O * I), None, op0=ALU.mult)
        nthr1 = statp.tile([1, 1], F32)
        nc.vector.tensor_scalar(nthr1[:], thr1[:], -1.0, None, op0=ALU.mult)
        thr128 = const.tile([P, 1], F32)
        nc.gpsimd.partition_broadcast(thr128[:], thr1[:])
        nthr128 = const.tile([P, 1], F32)
        nc.gpsimd.partition_broadcast(nthr128[:], nthr1[:])
        nc.sync.dma_start(sout[0:1, 0:1], gsum[:])
        nc.sync.dma_start(sout[0:1, 1:2], gmax[:])
        nc.sync.dma_start(sout[0:1, 2:3], sx1[:])

        # ---- Phase 2: exact fp8 pair split of Xq, interleaved with the
        # panel-0 W quant so the thr-gated ops fill DVE gaps ----
        # xpair[g] layout [p, c(4), plane(2), t(1024)]: plane 0 = Xc, 1 = R
        xpair = [None] * 8
        p0_quarters = [None] * 4
        for g in range(8):
            u = stgx.tile([P, GF], F32, tag="xstage")
            # u = round(x*sx) + MAGIC  (the fp32 add performs RNE rounding)
            nc.scalar.activation(u[:], xbf_tiles[g][:], IDENT,
                                 bias=pmagic128[:], scale=sx128[:])
            xp = xmem.tile([P, 2 * GF], F8E4, tag="xm", name=f"xp{g}")
            xp4 = xp[:].rearrange("p (c two t) -> p c two t", c=4, two=2)
            # Xc = e4m3(u - MAGIC): exact RNE cast of the integer Xq
            # (on DVE, chained with the stt: one cross-engine hop per group)
            nc.vector.tensor_scalar(
                xp4[:, :, 0, :], u[:].rearrange("p (c t) -> p c t", c=4),
                -MAGIC, None, op0=ALU.add,
            )
            # R = (u - MAGIC) - Xc: integer residual in [-4, 4]
            nc.vector.scalar_tensor_tensor(
                xp4[:, :, 1, :], u[:].rearrange("p (c t) -> p c t", c=4),
                MAGIC, xp4[:, :, 0, :], op0=ALU.subtract, op1=ALU.subtract,
            )
            xpair[g] = xp4
            if g % 2 == 1:
                # one panel-0 quarter after every second group: its two ops
                # park in the wait queue until thr fires, without ever
                # filling the 4-deep queue and stalling the X chain
                q = g // 2
                p0_quarters[q] = quarter_quant(0, q, p0_tiles[q])

        panel_quants = [p0_quarters, [
            quarter_quant(1, q, quarter_dma(1, q, engine=nc.sync))
            for q in range(4)
        ]]

        # ---- Phase 3: DoubleRow GEMM, quant pipelined ahead of the PE ----
        for op_ in range(8):
            quarters = panel_quants[op_]
            # 8 PSUM banks: (oc 0..3) x (tb 0..1); each accumulates all 32
            # i-chunks of this panel via DoubleRow matmuls
            for oc in range(4):
                for tb in range(2):
                    bank = oc * 2 + tb
                    ps = psum.tile([P, NMM], F32, tag=f"ps{bank}", name=f"ps{op_}_{bank}")
                    for ic in range(32):
                        q, sub = ic // 8, ic % 8
                        g, c = ic // 4, ic % 4
                        lhsT = (
                            quarters[q][:, sub, oc * P : (oc + 1) * P]
                            .unsqueeze(1)
                            .broadcast_to([P, 2, P])
                        )
                        rhs = xpair[g][:, c, :, _ts(tb, NMM)]
                        nc.tensor.matmul(
                            ps[:], lhsT=lhsT, rhs=rhs,
                            start=(ic == 0), stop=(ic == 31),
                            perf_mode=mybir.MatmulPerfMode.DoubleRow,
                        )
                    ot = osb.tile([P, NMM], BF16)
                    nc.scalar.copy(ot[:], ps[:])
                    # chunk (op_, oc, tb): rows = o-partitions, cols = t
                    nc.sync.dma_start(out[_ts(op_ * 8 + oc * 2 + tb, P), :], ot[:])
            if op_ < 6:
                panel_quants.append([
                    quarter_quant(op_ + 2, q, quarter_dma(op_ + 2, q))
                    for q in range(4)
                ])

        # ---- finalize nonzero count: nnz = sum(Wq) + 2*sum(b2) ----
        qacc_c = statp.tile([P, 1], F32)
        nc.vector.tensor_reduce(qacc_c[:], qaccs[:], axis=AXX.X, op=ALU.add)
        nacc_c = statp.tile([P, 1], F32)
        nc.vector.tensor_reduce(nacc_c[:], naccs[:], axis=AXX.X, op=ALU.add)
        nnz_c = statp.tile([P, 1], F32)
        nc.vector.scalar_tensor_tensor(
            nnz_c[:], nacc_c[:], 2.0, qacc_c[:], op0=ALU.mult, op1=ALU.add
        )
        nnz_a = statp.tile([P, 1], F32)
        nc.gpsimd.partition_all_reduce(
            nnz_a[:], nnz_c[:], channels=P, reduce_op=bass_isa.ReduceOp.add
        )
        nc.sync.dma_start(sout[0:1, 3:4], nnz_a[0:1, 0:1])


def _build():
    nc = bacc.Bacc("TRN2", debug=False, enable_asserts=False, num_devices=NC)
    xT_ap = nc.dram_tensor("xT_shard", (I, TSH), F32, kind="ExternalInput").ap()
    wT_ap = nc.dram_tensor("wT_full", (I, O), F32, kind="ExternalInput").ap()
    wsl_ap = nc.dram_tensor("wT_slice", (ISL, O), F32, kind="ExternalInput").ap()
    # chunked layout: row (panel*8 + oc*2 + tb)*128 + p, col c
    #   <->  outT[o = panel*512 + oc*128 + p, t = tb*512 + c]
    out_ap = nc.dram_tensor("out_shard", (64 * P, NMM), BF16, kind="ExternalOutput").ap()
    st_ap = nc.dram_tensor("stats_out", (1, 4), F32, kind="ExternalOutput").ap()
    with tile.TileContext(nc) as tc:
        _bitlinear(tc, out_ap, st_ap, xT_ap, wT_ap, wsl_ap)
    nc.compile()
    return nc


_NC_CACHE = None


def _get_nc():
    global _NC_CACHE
    if _NC_CACHE is None:
        _NC_CACHE = _build()
    return _NC_CACHE


def _run(x, weight, **spmd_kwargs):
    x = np.ascontiguousarray(np.asarray(x, dtype=np.float32))
    w = np.asarray(weight, dtype=np.float32)
    assert x.shape == (T, I) and w.shape == (O, I)
    nc = _get_nc()
    wT = np.ascontiguousarray(w.T)  # [I, O]
    in_maps = [
        {
            "xT_shard": np.ascontiguousarray(x[k * TSH : (k + 1) * TSH].T),
            "wT_full": wT,
            "wT_slice": wT[k * ISL : (k + 1) * ISL],  # contiguous view
        }
        for k in range(NC)
    ]
    res = run_bass_kernel_spmd(nc, in_maps, core_ids=list(range(NC)), **spmd_kwargs)
    outs = res.results

    st0 = outs[0]["stats_out"][0]
    gsum, sx = float(st0[0]), float(st0[2])
    nnz = float(st0[3])  # every core computed the exact global count

    # replicate the reference's fp32 scalar arithmetic
    f32 = np.float32
    n_el = f32(float(O) * float(I))
    abs_mean = f32(f32(gsum) / n_el)
    non_zero_mean = f32(f32(f32(nnz) / n_el) + f32(1e-8))
    scale_w = f32(abs_mean / non_zero_mean)
    scale = f32(np.float64(scale_w) / np.float64(sx))

    # un-chunk each core's [(panel,oc,tb)][128 o][512 t] output (transposed)
    out = np.empty((T, O), dtype=np.float32)
    for k in range(NC):
        chunk = outs[k]["out_shard"].astype(np.float32).reshape(8, 4, 2, P, NMM)
        # chunk[panel, oc, tb, p, c] = outT[panel*512 + oc*128 + p, tb*512 + c]
        shard_oT = chunk.transpose(0, 1, 3, 2, 4).reshape(O, TSH)
        out[k * TSH : (k + 1) * TSH] = shard_oT.T
    out *= scale
    return out, res


def kernel(x, weight):
    out, _ = _run(x, weight)
    return out


# revision 22
# speedup vs baseline: 1.2601x; 1.2601x over previous
"""BitLinear fake-quant GEMM on 8 TRN2 NeuronCores — fp8 DoubleRow edition.

Reference math:
  abs_mean  = mean(|W|);  thr = 0.7*abs_mean
  Wq        = sign(W) * (|W| >= thr)            (ternary)
  scale_w   = abs_mean / (mean(Wq != 0) + 1e-8)
  sx        = 127 / max(|X|)
  Xq        = round(X * sx)                      (integer valued, |.| <= 127)
  out       = (Xq @ Wq^T) * scale_w / sx

Sharding: data-parallel over tokens (8192/8 = 1024 token columns per core);
W is replicated.  The GEMM runs on the tensor engine in fp8e4 DoubleRow
perf mode, which fuses two K=128 plane-products into one matmul at half
the per-row cost.  The two planes carry an EXACT hi/lo split of the
integer activations:

  Xc = e4m3(Xq)          (RNE cast of the integer; |Xq| <= 127 < 240)
  R  = Xq - Xc           (integer residual, |R| <= 4 — exactly fp8)

so with the stationary weight plane pair (Wq, Wq) — a stride-0 broadcast,
no duplication — each DoubleRow matmul computes
  psum[o, t] += sum_i Wq[i,o]*(Xc[i,t] + R[i,t]) = sum_i Wq[i,o]*Xq[i,t]
exactly, in integers.

X is read once in fp32 (feeding the exact |x| max), cast to bf16 and kept
resident in SBUF; quantization re-reads it from SBUF.  The bf16 staging
perturbs round(x*sx) on ~2% of elements by +-1 (output max err ~2-3 vs a
tolerance of ~11), and the output is stored as bf16 (integer-valued sums
up to ~7000, max err ~1).  Everything else is exact.

Scheduling notes (engine SEQs are in-order with a 4-deep wait queue, and
the DMA device is a single 360 GB/s FIFO ordered by request time):
  SP   : x DMAs, then the collective staging hops (cin/gg) and stats
         outputs, then the out-chunk stores.  Nothing bulky is allowed to
         queue ahead of the staging hops.
  ACT  : x bf16 casts -> wsl DMA issues (their requests land right after
         the x stream drains) -> u = Id(sx*xbf + MAGIC) + half the Xc
         casts -> psum->bf16 output copies.
  Pool : x-max allreduce + AllGather (sx), panel-0 W-quarter DMA issues,
         w allreduce + AllGather (thr), panel 1+ W-quarter DMA issues
         (all W streaming stays off the gated SP/ACT queues).
  DVE  : x-max reduces, wsl reduces, X pair split (stt + half the Xc
         casts), W quant (b2 + signed combine) pipelined ahead of the PE.

Stats: each core reduces its own x shard and a distinct 512-row slice of
W^T; two tiny AllGathers + local reduce replace the global mean/max
all-reduces.  nnz falls out of the W-quant DVE ops' accum_out side-sums
(every core sees the full W).  The final scalar rescale by scale_w/sx is
applied on the host during the unshard, together with the [o, t] ->
[t, o] transpose of each shard.
"""

from contextlib import ExitStack

import numpy as np

import concourse.bass as bass
import concourse.bass_isa as bass_isa
import concourse.tile as tile
from concourse import bacc, mybir
from concourse.bass import ts as _ts
from concourse.bass_utils import run_bass_kernel_spmd

P = 128
T, I, O = 8192, 4096, 4096  # tokens, in_features, out_features
NC = 8
TSH = T // NC  # 1024 token columns per core
ISL = I // NC  # 512 wT rows per core for stats
NMM = 512  # matmul moving free dim (one fp32 PSUM bank)
GF = 4096  # streaming tile free size (one [128, 4096] fp32 tile = 2 MB)
MAGIC = 12582912.0  # 1.5 * 2**23: fp32 round-to-nearest-even bias trick

F32 = mybir.dt.float32
BF16 = mybir.dt.bfloat16
F8E4 = mybir.dt.float8e4
ALU = mybir.AluOpType
AXX = mybir.AxisListType
IDENT = mybir.ActivationFunctionType.Identity


def _bitlinear(tc, out, sout, xT, wT, wsl):
    nc = tc.nc
    with ExitStack() as ctx:
        const = ctx.enter_context(tc.tile_pool(name="const", bufs=1))
        statp = ctx.enter_context(tc.tile_pool(name="statp", bufs=1))
        dram = ctx.enter_context(tc.tile_pool(name="dram", bufs=1, space="DRAM"))
        stgx = ctx.enter_context(tc.tile_pool(name="stgx", bufs=2))   # f32 [128,4096]
        # shared pool: 8 resident bf16 x tiles, then 8 fp8 (Xc|R) pair tiles
        # reuse the freed slots (all 8 KB/partition)
        xmem = ctx.enter_context(tc.tile_pool(name="xmem", bufs=8))
        stgw = ctx.enter_context(tc.tile_pool(name="stgw", bufs=4))   # f32 [128,4096]
        wslp = ctx.enter_context(tc.tile_pool(name="wslp", bufs=2))   # f32 [128,1024]
        b2p = ctx.enter_context(tc.tile_pool(name="b2p", bufs=1))     # f8 [128,4096]
        wqp = ctx.enter_context(tc.tile_pool(name="wqp", bufs=2))     # 4x f8 [128,4096]
        psum = ctx.enter_context(tc.tile_pool(name="psum", bufs=1, space="PSUM"))
        osb = ctx.enter_context(tc.tile_pool(name="osb", bufs=2))     # bf16 [128,512]

        pmagic128 = const.tile([P, 1], F32)
        nc.gpsimd.memset(pmagic128[:], MAGIC)
        nmagic128 = const.tile([P, 1], F32)
        nc.gpsimd.memset(nmagic128[:], -MAGIC)

        # ---- Phase 1a: x stream (SP), stats reduces (DVE), bf16 casts (ACT)
        xmax_part = statp.tile([P, 8], F32)
        xbf_tiles = [None] * 8
        for g in range(8):
            xt = stgx.tile([P, GF], F32, tag="xstage")
            src = xT[g * 512 : (g + 1) * 512, :].rearrange("(c p) t -> p c t", p=P)
            nc.sync.dma_start(xt[:].rearrange("p (c t) -> p c t", c=4), src)
            nc.vector.tensor_reduce(
                xmax_part[:, g : g + 1], xt[:], axis=AXX.X, op=ALU.max,
                apply_absolute_value=True,
            )
            xb = xmem.tile([P, GF], BF16, tag="xm", name=f"xb{g}")
            nc.scalar.copy(xb[:], xt[:])
            xbf_tiles[g] = xb

        # ---- Phase 1b: x-max collective staging.  The SP queue order is
        # [x DMAs, cin_x, wsl DMAs, ggx, cin_w, ggw]: the DMA device is a
        # FIFO ordered by request time, and each gated SP instruction
        # naturally holds the next bulk stream's requests back until the
        # preceding collective hop is in flight. ----
        xmax_c = statp.tile([P, 1], F32)
        nc.vector.tensor_reduce(xmax_c[:], xmax_part[:], axis=AXX.X, op=ALU.max)
        xmax_a = statp.tile([P, 1], F32)
        nc.gpsimd.partition_all_reduce(
            xmax_a[:], xmax_c[:], channels=P, reduce_op=bass_isa.ReduceOp.max
        )
        cin_x = dram.tile([1, 1], F32)
        cout_x = dram.tile([1, NC], F32)
        nc.sync.dma_start(cin_x[:], xmax_a[0:1, 0:1])
        nc.gpsimd.collective_compute(
            "AllGather", ALU.bypass, replica_groups=[list(range(NC))],
            ins=[cin_x.opt()], outs=[cout_x.opt()],
        )
        # wsl stream: SP issues these only after cin_x's wait fires, so
        # their requests queue behind the whole x stream + cin_x
        wsl_tiles = []
        for c in range(16):
            wt = wslp.tile([P, GF // 4], F32, tag="wslstage")
            nc.sync.dma_start(
                wt[:],
                wsl[(c // 4) * P : (c // 4 + 1) * P,
                    (c % 4) * (GF // 4) : (c % 4 + 1) * (GF // 4)],
            )
            wsl_tiles.append(wt)
        ggx = statp.tile([1, NC], F32)
        nc.sync.dma_start(ggx[:], cout_x[:])
        gmax = statp.tile([1, 1], F32)
        nc.vector.tensor_reduce(gmax[:], ggx[:], axis=AXX.X, op=ALU.max)
        gmax_c = statp.tile([1, 1], F32)
        nc.vector.tensor_scalar(gmax_c[:], gmax[:], 1e-12, None, op0=ALU.max)
        rec1 = statp.tile([1, 1], F32)
        nc.vector.reciprocal(rec1[:], gmax_c[:])
        sx1 = statp.tile([1, 1], F32)
        nc.vector.tensor_scalar(sx1[:], rec1[:], 127.0, None, op0=ALU.mult)
        sx128 = const.tile([P, 1], F32)
        nc.gpsimd.partition_broadcast(sx128[:], sx1[:])

        # ---- W quarter streaming (DMA on the Pool queue; quant on DVE) ----
        qaccs = statp.tile([P, 32], F32)  # sum(Wq) per quarter  ( #pos - #neg )
        naccs = statp.tile([P, 32], F32)  # sum(b2) per quarter  ( #neg )

        def quarter_dma(op_, q, engine=None):
            wt = stgw.tile([P, GF], F32, tag="wstage")
            src_ = wT[
                q * 1024 : (q + 1) * 1024, _ts(op_, NMM)
            ].rearrange("(c p) j -> p c j", p=P)
            (engine or nc.gpsimd).dma_start(
                wt[:].rearrange("p (c j) -> p c j", c=8), src_)
            return wt

        def quarter_quant(op_, q, wt):
            col = op_ * 4 + q
            b2 = b2p.tile([P, GF], F8E4, tag="b2")
            nc.vector.tensor_scalar(
                b2[:], wt[:], nthr128[:], None, op0=ALU.is_le, op1=ALU.add,
                accum_out=naccs[:, col : col + 1],
            )
            wq = wqp.tile([P, GF], F8E4, tag=f"wq{q}")
            nc.vector.scalar_tensor_tensor(
                wq[:], wt[:], thr128[:], b2[:],
                op0=ALU.is_ge, op1=ALU.subtract,
                accum_out=qaccs[:, col : col + 1],
            )
            return wq[:].rearrange("p (c j) -> p c j", c=8)

        # ---- wsl reduces (DVE) + W-sum collective.  Pool order:
        # [allred-x, allred-w, P0 quarter DMAs, bcast-sx, bcast-thr...]:
        # the P0 DMA issues fire right after allred-w (~when the wsl
        # stream drains), so their requests land between cin_w and the
        # Pool-streamed panel-1+ quarters. ----
        wsum_part = statp.tile([P, 16], F32)
        for c in range(16):
            nc.vector.tensor_reduce(
                wsum_part[:, c : c + 1], wsl_tiles[c][:], axis=AXX.X, op=ALU.add,
                apply_absolute_value=True,
            )
        wsum_c = statp.tile([P, 1], F32)
        nc.vector.tensor_reduce(wsum_c[:], wsum_part[:], axis=AXX.X, op=ALU.add)
        wsum_a = statp.tile([P, 1], F32)
        nc.gpsimd.partition_all_reduce(
            wsum_a[:], wsum_c[:], channels=P, reduce_op=bass_isa.ReduceOp.add
        )
        cin_w = dram.tile([1, 1], F32)
        cout_w = dram.tile([1, NC], F32)
        nc.sync.dma_start(cin_w[:], wsum_a[0:1, 0:1])
        nc.gpsimd.collective_compute(
            "AllGather", ALU.bypass, replica_groups=[list(range(NC))],
            ins=[cin_w.opt()], outs=[cout_w.opt()],
        )
        ggw = statp.tile([1, NC], F32)
        nc.sync.dma_start(ggw[:], cout_w[:])
        p0_tiles = [quarter_dma(0, q, engine=nc.sync) for q in range(4)]
        gsum = statp.tile([1, 1], F32)
        nc.vector.tensor_reduce(gsum[:], ggw[:], axis=AXX.X, op=ALU.add)
        thr1 = statp.tile([1, 1], F32)
        nc.vector.tensor_scalar(thr1[:], gsum[:], 0.7 / float(O * I), None, op0=ALU.mult)
        nthr1 = statp.tile([1, 1], F32)
        nc.vector.tensor_scalar(nthr1[:], thr1[:], -1.0, None, op0=ALU.mult)
        thr128 = const.tile([P, 1], F32)
        nc.gpsimd.partition_broadcast(thr128[:], thr1[:])
        nthr128 = const.tile([P, 1], F32)
        nc.gpsimd.partition_broadcast(nthr128[:], nthr1[:])
        nc.sync.dma_start(sout[0:1, 0:1], gsum[:])
        nc.sync.dma_start(sout[0:1, 1:2], gmax[:])
        nc.sync.dma_start(sout[0:1, 2:3], sx1[:])

        # ---- Phase 2: exact fp8 pair split of Xq, interleaved with the
        # panel-0 W quant so the thr-gated ops fill DVE gaps ----
        # xpair[g] layout [p, c(4), plane(2), t(1024)]: plane 0 = Xc, 1 = R
        xpair = [None] * 8
        p0_quarters = [None] * 4
        for g in range(8):
            u = stgx.tile([P, GF], F32, tag="xstage")
            # u = round(x*sx) + MAGIC  (the fp32 add performs RNE rounding)
            nc.scalar.activation(u[:], xbf_tiles[g][:], IDENT,
                                 bias=pmagic128[:], scale=sx128[:])
            xp = xmem.tile([P, 2 * GF], F8E4, tag="xm", name=f"xp{g}")
            xp4 = xp[:].rearrange("p (c two t) -> p c two t", c=4, two=2)
            # Xc = e4m3(u - MAGIC): exact RNE cast of the integer Xq
            # (on DVE, chained with the stt: one cross-engine hop per group)
            nc.vector.tensor_scalar(
                xp4[:, :, 0, :], u[:].rearrange("p (c t) -> p c t", c=4),
                -MAGIC, None, op0=ALU.add,
            )
            # R = (u - MAGIC) - Xc: integer residual in [-4, 4]
            nc.vector.scalar_tensor_tensor(
                xp4[:, :, 1, :], u[:].rearrange("p (c t) -> p c t", c=4),
                MAGIC, xp4[:, :, 0, :], op0=ALU.subtract, op1=ALU.subtract,
            )
            xpair[g] = xp4
            if g % 2 == 1:
                # one panel-0 quarter after every second group: its two ops
                # park in the wait queue until thr fires, without ever
                # filling the 4-deep queue and stalling the X chain
                q = g // 2
                p0_quarters[q] = quarter_quant(0, q, p0_tiles[q])

        panel_quants = [p0_quarters, [
            quarter_quant(1, q, quarter_dma(1, q, engine=nc.sync))
            for q in range(4)
        ]]

        # ---- Phase 3: DoubleRow GEMM, quant pipelined ahead of the PE ----
        for op_ in range(8):
            quarters = panel_quants[op_]
            # 8 PSUM banks: (oc 0..3) x (tb 0..1); each accumulates all 32
            # i-chunks of this panel via DoubleRow matmuls
            for oc in range(4):
                for tb in range(2):
                    bank = oc * 2 + tb
                    ps = psum.tile([P, NMM], F32, tag=f"ps{bank}", name=f"ps{op_}_{bank}")
                    for ic in range(32):
                        q, sub = ic // 8, ic % 8
                        g, c = ic // 4, ic % 4
                        lhsT = (
                            quarters[q][:, sub, oc * P : (oc + 1) * P]
                            .unsqueeze(1)
                            .broadcast_to([P, 2, P])
                        )
                        rhs = xpair[g][:, c, :, _ts(tb, NMM)]
                        nc.tensor.matmul(
                            ps[:], lhsT=lhsT, rhs=rhs,
                            start=(ic == 0), stop=(ic == 31),
                            perf_mode=mybir.MatmulPerfMode.DoubleRow,
                        )
                    ot = osb.tile([P, NMM], BF16)
                    nc.scalar.copy(ot[:], ps[:])
                    # chunk (op_, oc, tb): rows = o-partitions, cols = t
                    nc.sync.dma_start(out[_ts(op_ * 8 + oc * 2 + tb, P), :], ot[:])
            if op_ < 6:
                panel_quants.append([
                    quarter_quant(op_ + 2, q, quarter_dma(op_ + 2, q))
                    for q in range(4)
                ])

        # ---- finalize nonzero count: nnz = sum(Wq) + 2*sum(b2) ----
        qacc_c = statp.tile([P, 1], F32)
        nc.vector.tensor_reduce(qacc_c[:], qaccs[:], axis=AXX.X, op=ALU.add)
        nacc_c = statp.tile([P, 1], F32)
        nc.vector.tensor_reduce(nacc_c[:], naccs[:], axis=AXX.X, op=ALU.add)
        nnz_c = statp.tile([P, 1], F32)
        nc.vector.scalar_tensor_tensor(
            nnz_c[:], nacc_c[:], 2.0, qacc_c[:], op0=ALU.mult, op1=ALU.add
        )
        nnz_a = statp.tile([P, 1], F32)
        nc.gpsimd.partition_all_reduce(
            nnz_a[:], nnz_c[:], channels=P, reduce_op=bass_isa.ReduceOp.add
        )
        nc.sync.dma_start(sout[0:1, 3:4], nnz_a[0:1, 0:1])


def _build():
    nc = bacc.Bacc("TRN2", debug=False, enable_asserts=False, num_devices=NC)
    xT_ap = nc.dram_tensor("xT_shard", (I, TSH), F32, kind="ExternalInput").ap()
    wT_ap = nc.dram_tensor("wT_full", (I, O), F32, kind="ExternalInput").ap()
    wsl_ap = nc.dram_tensor("wT_slice", (ISL, O), F32, kind="ExternalInput").ap()
    # chunked layout: row (panel*8 + oc*2 + tb)*128 + p, col c
    #   <->  outT[o = panel*512 + oc*128 + p, t = tb*512 + c]
    out_ap = nc.dram_tensor("out_shard", (64 * P, NMM), BF16, kind="ExternalOutput").ap()
    st_ap = nc.dram_tensor("stats_out", (1, 4), F32, kind="ExternalOutput").ap()
    with tile.TileContext(nc) as tc:
        _bitlinear(tc, out_ap, st_ap, xT_ap, wT_ap, wsl_ap)
    nc.compile()
    return nc


_NC_CACHE = None


def _get_nc():
    global _NC_CACHE
    if _NC_CACHE is None:
        _NC_CACHE = _build()
    return _NC_CACHE


def _run(x, weight, **spmd_kwargs):
    x = np.ascontiguousarray(np.asarray(x, dtype=np.float32))
    w = np.asarray(weight, dtype=np.float32)
    assert x.shape == (T, I) and w.shape == (O, I)
    nc = _get_nc()
    wT = np.ascontiguousarray(w.T)  # [I, O]
    in_maps = [
        {
            "xT_shard": np.ascontiguousarray(x[k * TSH : (k + 1) * TSH].T),
            "wT_full": wT,
            "wT_slice": wT[k * ISL : (k + 1) * ISL],  # contiguous view
        }
        for k in range(NC)
    ]
    res = run_bass_kernel_spmd(nc, in_maps, core_ids=list(range(NC)), **spmd_kwargs)
    outs = res.results

    st0 = outs[0]["stats_out"][0]
    gsum, sx = float(st0[0]), float(st0[2])
    nnz = float(st0[3])  # every core computed the exact global count

    # replicate the reference's fp32 scalar arithmetic
    f32 = np.float32
    n_el = f32(float(O) * float(I))
    abs_mean = f32(f32(gsum) / n_el)
    non_zero_mean = f32(f32(f32(nnz) / n_el) + f32(1e-8))
    scale_w = f32(abs_mean / non_zero_mean)
    scale = f32(np.float64(scale_w) / np.float64(sx))

    # un-chunk each core's [(panel,oc,tb)][128 o][512 t] output (transposed)
    out = np.empty((T, O), dtype=np.float32)
    for k in range(NC):
        chunk = outs[k]["out_shard"].astype(np.float32).reshape(8, 4, 2, P, NMM)
        # chunk[panel, oc, tb, p, c] = outT[panel*512 + oc*128 + p, tb*512 + c]
        shard_oT = chunk.transpose(0, 1, 3, 2, 4).reshape(O, TSH)
        out[k * TSH : (k + 1) * TSH] = shard_oT.T
    out *= scale
    return out, res


def kernel(x, weight):
    out, _ = _run(x, weight)
    return out
